# revision 1
# baseline (speedup 1.0000x reference)
"""Trainium2 Bass kernel for a 4-head spatial MultiHeadAttention block.

Reference computation (per batch n):
    q/k/v = 1x1-conv projections of x (C=256 channels, S=48*48=2304 positions)
    per head (4 heads, d=64): attn = softmax(q^T k / 8), out = attn @ v
    out = Wo @ concat(heads) + bo + x   (residual)

Sharding across 8 NeuronCores: core c handles batch n = c//2 and head-pair
hp = c%2 (output channels [hp*128, hp*128+128) of the QKV projections, i.e.
heads {2*hp, 2*hp+1}).  Each core computes a partial output
Wo[:, ch] @ attn_ch (256 x 2304); the host sums the two partials per batch
and adds bo + residual x.

Per-core kernel layout choices:
  - Q stored (d, s), d on partitions: rows 0-63 head A, 64-127 head B.
  - K stored zero-padded per head (Kz0: head A rows + zero rows, Kz1: head B
    rows + zero rows) so every scores matmul contracts the full 128
    partitions and all attention matmuls share one PE tile config
    (128x128) - PE tile-config switches cost ~150ns per matmul.
  - V is produced transposed (VT: t on partitions, d on free) directly by the
    projection matmul, with a constant-1 column appended per head so the
    attn@V matmul also yields the softmax row-sums for free (M=65).
  - scoresT(t,s) = Kz_h^T Q; 3 t-tiles are packed into one 3-bank PSUM tile
    so exp (ScalarE) runs on 1536-wide batches straight out of PSUM.
  - software pipeline: the attn@V matmuls of exp-batch g are emitted after
    the scores matmuls of batch g+1, so the PE never waits on ScalarE.
  - normalization: reciprocal on a (64, sw/64) lane-spread reshape (a plain
    (1, sw) reciprocal runs on a single DVE lane at 8 cycles/element), then
    partition-broadcast via a DRAM bounce.
All matmul operands are bf16; accumulation and softmax math are fp32.
"""

import numpy as np

import concourse.bass as bass
import concourse.mybir as mybir
import concourse.tile as tile
from concourse import bacc
from concourse.bass_utils import run_bass_kernel_spmd

C = 256          # channels
S = 2304         # spatial positions (48*48)
HD = 64          # head dim
P = 128          # partitions
TT = S // P      # 18 t-tiles of 128
GRP = 3          # t-tiles per exp batch (3 PSUM banks)
SCALE = 0.125    # 1/sqrt(HD)
F32 = mybir.dt.float32
BF16 = mybir.dt.bfloat16

S_CHUNKS = [(0, 512), (512, 512), (1024, 512), (1536, 512), (2048, 256)]


def _body(tc):
    nc = tc.nc
    t_x = nc.dram_tensor("x", [C, S], BF16, kind="ExternalInput").ap()
    t_wqt = nc.dram_tensor("wqt", [C, P], BF16, kind="ExternalInput").ap()
    t_wkt = nc.dram_tensor("wkt", [C, P], BF16, kind="ExternalInput").ap()
    t_wvt = nc.dram_tensor("wvt", [C, P], BF16, kind="ExternalInput").ap()
    t_wot = nc.dram_tensor("wot", [P, C], BF16, kind="ExternalInput").ap()
    t_bq = nc.dram_tensor("bq", [P, 1], F32, kind="ExternalInput").ap()
    t_bk = nc.dram_tensor("bk", [P, 1], F32, kind="ExternalInput").ap()
    t_bv = nc.dram_tensor("bv", [1, P], F32, kind="ExternalInput").ap()
    t_out = nc.dram_tensor("out", [C, S], F32, kind="ExternalOutput").ap()

    singles = tc.alloc_tile_pool(name="singles", bufs=1)
    x_lo = singles.tile([P, S], BF16)
    x_hi = singles.tile([P, S], BF16)
    q_sb = singles.tile([P, S], BF16)
    kz0 = singles.tile([P, S], BF16)          # head A rows 0-63, zeros 64-127
    kz1 = singles.tile([P, S], BF16)          # zeros 0-63, head B rows 64-127
    vt_sb = singles.tile([P, TT, 130], BF16)  # per tt: [dA(64) | 1 | dB(64) | 1]
    wq_sb = singles.tile([P, 2, P], BF16)
    wk_sb = singles.tile([P, 2, P], BF16)
    wv_sb = singles.tile([P, 2, P], BF16)
    wot_sb = singles.tile([P, C], BF16)
    attn_full = singles.tile([P, S], BF16)
    bq_sb = singles.tile([P, 1], F32)
    bk_sb = singles.tile([P, 1], F32)
    bv_bc = singles.tile([P, P], F32)

    # ---- input DMAs: weights first (tiny), then x split across two queues ----
    nc.sync.dma_start(out=wk_sb, in_=t_wkt.rearrange("(a p) d -> p a d", p=P))
    nc.sync.dma_start(out=x_lo[:, 0:512], in_=t_x[0:P, 0:512])
    nc.sync.dma_start(out=x_hi[:, 0:512], in_=t_x[P:C, 0:512])
    nc.gpsimd.dma_start(out=bk_sb, in_=t_bk)
    nc.gpsimd.dma_start(out=bq_sb, in_=t_bq)
    nc.sync.dma_start(out=wq_sb, in_=t_wqt.rearrange("(a p) d -> p a d", p=P))
    nc.gpsimd.dma_start(out=wv_sb, in_=t_wvt.rearrange("(a p) d -> p a d", p=P))
    nc.gpsimd.dma_start(out=bv_bc, in_=t_bv.to_broadcast([P, P]))
    nc.gpsimd.dma_start(out=wot_sb, in_=t_wot)
    for ci, (s0, sw) in enumerate(S_CHUNKS[1:]):
        eng = nc.sync if ci % 2 == 0 else nc.gpsimd
        eng.dma_start(out=x_lo[:, s0:s0 + sw], in_=t_x[0:P, s0:s0 + sw])
        eng.dma_start(out=x_hi[:, s0:s0 + sw], in_=t_x[P:C, s0:s0 + sw])
    # zero the dead half of each Kz; ones-columns (64, 129) of vt survive the
    # per-tile evictions which overwrite all other columns
    nc.vector.memset(kz0[HD:P, :], 0.0)
    nc.vector.memset(kz1[0:HD, :], 0.0)
    nc.vector.memset(vt_sb[:, :, :], 1.0)

    ps = tc.alloc_tile_pool(name="ps", bufs=2, space="PSUM")
    ex_pool = tc.alloc_tile_pool(name="ex_sb", bufs=4)
    nrm = tc.alloc_tile_pool(name="nrm", bufs=2)
    wo_out = tc.alloc_tile_pool(name="wo_out", bufs=4)
    sdram = tc.alloc_tile_pool(name="sdram", bufs=2, space="DRAM")

    def k_chunk(s0, sw):
        psn = ps.tile([P, GRP * 512], F32, tag="sc", name="kps")[:, :sw]
        nc.tensor.matmul(psn, wk_sb[:, 0, :], x_lo[:, s0:s0 + sw],
                         start=True, stop=False)
        nc.tensor.matmul(psn, wk_sb[:, 1, :], x_hi[:, s0:s0 + sw],
                         start=False, stop=True)
        nc.vector.tensor_scalar_add(kz0[0:HD, s0:s0 + sw], psn[0:HD, :],
                                    bk_sb[0:HD, :])
        nc.vector.tensor_scalar_add(kz1[HD:P, s0:s0 + sw], psn[HD:P, :],
                                    bk_sb[HD:P, :])

    def q_chunk(s0, sw):
        psn = ps.tile([P, GRP * 512], F32, tag="sc", name="qps")[:, :sw]
        nc.tensor.matmul(psn, wq_sb[:, 0, :], x_lo[:, s0:s0 + sw],
                         start=True, stop=False)
        nc.tensor.matmul(psn, wq_sb[:, 1, :], x_hi[:, s0:s0 + sw],
                         start=False, stop=True)
        nc.vector.tensor_scalar_add(q_sb[:, s0:s0 + sw], psn, bq_sb)

    def vt_tiles(tts):
        for tt in tts:
            psn = ps.tile([P, GRP * 512], F32, tag="sc", name="vtps")[:, :P]
            nc.tensor.matmul(psn, x_lo[:, tt * P:(tt + 1) * P], wv_sb[:, 0, :],
                             start=True, stop=False)
            nc.tensor.matmul(psn, x_hi[:, tt * P:(tt + 1) * P], wv_sb[:, 1, :],
                             start=False, stop=True)
            nc.vector.tensor_add(vt_sb[:, tt, 0:HD], psn[:, 0:HD], bv_bc[:, 0:HD])
            nc.vector.tensor_add(vt_sb[:, tt, 65:65 + HD], psn[:, HD:P],
                                 bv_bc[:, HD:P])

    def emit_av(pend):
        ex, g, ot, h, sw = pend
        for j in range(GRP):
            tt = g * GRP + j
            nc.tensor.matmul(ot, vt_sb[:, tt, h * 65:(h + 1) * 65],
                             ex[:, j * sw:(j + 1) * sw],
                             start=(tt == 0), stop=(tt == TT - 1))

    def wo_chunk(s0, sw):
        for half in range(2):
            psn = ps.tile([P, GRP * 512], F32, tag="sc", name="wops")[:, :sw]
            nc.tensor.matmul(psn, wot_sb[:, half * P:(half + 1) * P],
                             attn_full[:, s0:s0 + sw], start=True, stop=True)
            ob = wo_out.tile([P, 512], F32, tag="ob", name="ob")[:, :sw]
            nc.vector.tensor_copy(ob, psn)
            nc.sync.dma_start(out=t_out[half * P:(half + 1) * P, s0:s0 + sw],
                              in_=ob)

    def emit_norm(ot, h, s0, sw):
        comb = nrm.tile([65, 512], F32, tag="comb", name="comb")[:, :sw]
        nc.vector.tensor_copy(comb, ot)
        w8 = sw // HD  # elements per lane after the (64, w8) spread
        rs = nrm.tile([HD, 8], F32, tag="rs", name="rs")[:, :w8]
        nc.sync.dma_start(out=rs, in_=comb[HD:HD + 1, :])
        rr = nrm.tile([HD, 8], F32, tag="rr", name="rr")[:, :w8]
        nc.vector.reciprocal(rr, rs)
        lin = nrm.tile([1, 512], F32, tag="lin", name="lin")[:, :sw]
        nc.sync.dma_start(out=lin, in_=rr)
        rb = nrm.tile([HD, 512], F32, tag="rb", name="rb")[:, :sw]
        lin_bc = bass.AP(tensor=lin.tensor, offset=lin.offset,
                         ap=[lin.ap[0], [0, HD], lin.ap[1]])
        nc.sync.dma_start(out=rb, in_=lin_bc)
        if h == 0:
            nc.vector.tensor_mul(attn_full[0:HD, s0:s0 + sw], comb[0:HD, :], rb)
        else:
            a1 = nrm.tile([HD, 512], BF16, tag="a1", name="a1")[:, :sw]
            nc.vector.tensor_mul(a1, comb[0:HD, :], rb)
            nc.sync.dma_start(out=attn_full[HD:P, s0:s0 + sw], in_=a1)

    # ---- projections needed before the first exp batch ----
    for s0, sw in S_CHUNKS:
        k_chunk(s0, sw)
    q_chunk(*S_CHUNKS[0])

    # ---- attention: software-pipelined across all (s-chunk, head) units ----
    pend = None       # (ex, g, ot, h, sw): exp batch whose attn@V is pending
    pend_norm = None  # (ot, h, s0, sw): unit awaiting normalization
    wo_q = []         # (unit_idx, s0, sw) chunks whose Wo projection is pending
    weave = 0         # startup weave: VT + remaining Q between early exps
    unit = 0
    for s0, sw in S_CHUNKS:
        for h in range(2):
            unit += 1
            kz = kz0 if h == 0 else kz1
            ot = ps.tile([65, 512], F32, tag="ot", name="ot")[:, :sw]
            for g in range(TT // GRP):
                sc = ps.tile([P, GRP * 512], F32, tag="sc", name="sc")[:, :GRP * sw]
                for j in range(GRP):
                    tt = g * GRP + j
                    nc.tensor.matmul(sc[:, j * sw:(j + 1) * sw],
                                     kz[:, tt * P:(tt + 1) * P],
                                     q_sb[:, s0:s0 + sw],
                                     start=True, stop=True)
                # startup weave, fine-grained: VT tiles 3 per group, emitted
                # just before the attn@V batch that reads them (Tile deps are
                # emission-ordered); remaining Q chunks one per group after VT
                if weave < TT // GRP and (s0, h) == (0, 0) and g >= 1:
                    vt_tiles(range(weave * GRP, (weave + 1) * GRP))
                    weave += 1
                elif weave < TT // GRP:  # (s0, h1): finish VT
                    vt_tiles(range(weave * GRP, (weave + 1) * GRP))
                    weave += 1
                elif weave < TT // GRP + len(S_CHUNKS) - 1:
                    q_chunk(*S_CHUNKS[weave - TT // GRP + 1])
                    weave += 1
                if pend is not None:
                    emit_av(pend)
                    if pend[1] == TT // GRP - 1:  # last batch of its unit
                        emit_norm(*pend_norm)
                        if pend_norm[1] == 1:  # head B done: queue Wo
                            wo_q.append((unit, pend_norm[2], pend_norm[3]))
                if wo_q and g == 2 and unit > wo_q[0][0] + 1:
                    # a full unit after its normalize chain started
                    wo_chunk(*wo_q.pop(0)[1:])
                ex = ex_pool.tile([P, GRP * 512], BF16, tag="ex", name="ex")[:, :GRP * sw]
                nc.scalar.activation(ex, sc, mybir.ActivationFunctionType.Exp,
                                     scale=SCALE)
                pend = (ex, g, ot, h, sw)
                if g == TT // GRP - 1:
                    pend_norm = (ot, h, s0, sw)
    emit_av(pend)
    emit_norm(*pend_norm)
    wo_q.append((unit, pend_norm[2], pend_norm[3]))
    for wq_item in wo_q:
        wo_chunk(*wq_item[1:])

    sdram.release()
    wo_out.release()
    nrm.release()
    ex_pool.release()
    ps.release()
    singles.release()


_NC_CACHE = {}


def build_nc():
    if "nc" not in _NC_CACHE:
        nc = bacc.Bacc("TRN2", target_bir_lowering=False, debug=False, num_devices=8)
        with tile.TileContext(nc) as tc:
            _body(tc)
        nc.compile()
        _NC_CACHE["nc"] = nc
    return _NC_CACHE["nc"]


def make_in_maps(x, Wq, bq, Wk, bk, Wv, bv, Wo, bo):
    import ml_dtypes
    bf16 = ml_dtypes.bfloat16
    N = x.shape[0]
    xf = np.ascontiguousarray(np.asarray(x, np.float32).reshape(N, C, S).astype(bf16))
    in_maps = []
    for c in range(8):
        n, hp = c // 2, c % 2
        ch = slice(hp * P, (hp + 1) * P)
        wot = np.ascontiguousarray(np.asarray(Wo, np.float32)[:, ch].T.astype(bf16))  # (128, 256)
        in_maps.append({
            "x": xf[n],
            "wqt": np.ascontiguousarray(np.asarray(Wq, np.float32)[ch].T.astype(bf16)),
            "wkt": np.ascontiguousarray(np.asarray(Wk, np.float32)[ch].T.astype(bf16)),
            "wvt": np.ascontiguousarray(np.asarray(Wv, np.float32)[ch].T.astype(bf16)),
            "wot": wot,
            "bq": np.ascontiguousarray(np.asarray(bq, np.float32)[ch].reshape(P, 1)),
            "bk": np.ascontiguousarray(np.asarray(bk, np.float32)[ch].reshape(P, 1)),
            "bv": np.ascontiguousarray(np.asarray(bv, np.float32)[ch].reshape(1, P)),
        })
    return in_maps


def run(inputs, **kwargs):
    """Run on 8 cores; returns (full output, BassKernelResults)."""
    nc = build_nc()
    in_maps = make_in_maps(**inputs)
    res = run_bass_kernel_spmd(nc, in_maps, core_ids=list(range(8)), **kwargs)
    x = np.asarray(inputs["x"], np.float32)
    bo = np.asarray(inputs["bo"], np.float32)
    N, _, H, W = x.shape
    out = np.empty((N, C, S), np.float32)
    for n in range(N):
        out[n] = (x[n].reshape(C, S)
                  + res.results[2 * n]["out"]
                  + res.results[2 * n + 1]["out"]
                  + bo[:, None])
    return out.reshape(N, C, H, W), res


def kernel(**inputs):
    out, _ = run(inputs)
    return out

